# revision 15
# baseline (speedup 1.0000x reference)
"""Trainium2 Bass kernel for nn_MemoryNetwork (GRU-style memory network scan).

Model (per reference):
  t_enc = cos(arange(T) * freq + phase)                    [T, D]
  s0 = mean_t(x)                                           [B*C, D]
  tr = arange(T) * mask; x_seq = x[tr]; te_seq = t_enc[tr]
  per step t:
    msg = gelu([x_t, s, te_t] @ msg_W.T + msg_b)
    gi = msg @ W_ih.T + b_ih ; gh = s @ W_hh.T + b_hh
    r = sigmoid(i_r + h_r); z = sigmoid(i_z + h_z)
    n = tanh(i_n + r * h_n)
    s' = (1 - z) * n + z * s
  output: states [T, B, C, D]

Strategy: data-parallel over B*C = 4096 rows -> 8 cores x 512 rows.
On-device layout is feature-major and PARTITION-PACKED: two rows share a
packed column (row c on partitions 0:64, row 256+c on partitions 64:128),
with block-diagonal doubled weights [128, 128].  Matmul cost depends only
on the free dim, so packing halves the activation-engine work (the
bottleneck) for free.  The 256 packed columns split into NPB=3
independent chains (86/85/85).

Engines execute their queues IN PROGRAM ORDER, so the emission order is
SOFTWARE-PIPELINED: chain p runs phase (slot - p) % 3 of its step at each
slot, staggering the three chains across the three ACT ops of a step
(gelu / zr-tanh / n-tanh).  Between two consecutive ACT ops of one chain
the ACT engine executes the other two chains' (independent) ops, hiding
each chain's PE/DVE dependency latency.  Phases:

  A(t): s-dependent matmuls (Ws@s into pm; Whz/Whr@s + z/r bias rows into
        zr; Whn@s + bias into hh; b_in into pw), then u = gelu(pm).
  B(t): u-side matmuls (Wiz/Wir@u, Win@u), hzr = tanh([-a_z/2 | +a_r/2]),
        q = (hr+1)*hh on DVE, I@q accumulated into pw (PE),
        Q = 0.5*hz+0.5, Q' = -0.5*hz+0.5 (= 1-z, z), w1 = Q'*s on DVE.
  C(t): n = tanh(pw), v = Q*n, s' = v + w1 (= (1-z)*n + z*s), Pool-copy
        s' to the output stage, and prehoist step t+1's x-dependent
        matmuls (tb broadcast row with start=True, Wx@x) into a fresh
        PSUM bank.

All gate scales/biases are folded into the doubled weights or added in
PSUM via broadcast-row matmuls, so every ACT op is plain func(x).  Each
chain-step owns one PSUM bank [128, 512] f32 (one start=True / one
stop=True per bank: start zeroes the whole 2KB zero-region).  Outputs
stream out as bf16 every CH steps; the host unpacks partitions to rows.
"""

import sys

import numpy as np

sys.path.insert(0, "/opt/trn_rl_repo")

import ml_dtypes  # noqa: E402

BF16 = ml_dtypes.bfloat16

T, B, C, D = 256, 64, 64, 64
NCORES = 8
ROWS = (B * C) // NCORES  # 512 rows per core
HALF = ROWS // 2  # 256 packed columns (2 rows per column)
CH = 8  # timesteps per DMA chunk
NPB = 3  # packed chains per core
PF = [86, 85, 85]  # packed columns per chain
POFF = [0, 86, 171, 256]

_PROGRAM_CACHE = {}


def _build_program():
    import concourse.bacc as bacc
    import concourse.tile as tile
    from concourse import mybir
    from contextlib import ExitStack

    BF = mybir.dt.bfloat16
    F32 = mybir.dt.float32
    AF = mybir.ActivationFunctionType
    OP = mybir.AluOpType

    nc = bacc.Bacc(None, target_bir_lowering=False, debug=False)

    xP = nc.dram_tensor("xP", [T, 2 * D, HALF], BF, kind="ExternalInput")
    s0 = nc.dram_tensor("s0", [2 * D, HALF], BF, kind="ExternalInput")
    tb = nc.dram_tensor("tb", [1, T, 2 * D], BF, kind="ExternalInput")
    # 8 block-diagonal doubled weight mats + identity, packed [128, 9*128]:
    #   wx, ws, wiz(-.5), whz(-.5), wir(+.5), whr(+.5), win, whn(.5), I
    wblob = nc.dram_tensor("wblob", [2 * D, 9 * 2 * D], BF, kind="ExternalInput")
    # bias rows on partition 0, side by side [1, 4*128]:
    # -0.5(b_iz+b_hz) | +0.5(b_ir+b_hr) | b_in | 0.5*b_hn  (each tiled 2x)
    bblob = nc.dram_tensor("bblob", [1, 4 * 2 * D], BF, kind="ExternalInput")
    outP = nc.dram_tensor("outP", [T, 2 * D, HALF], BF, kind="ExternalOutput")

    DD = 2 * D  # 128

    with ExitStack() as ctx:
        tc = ctx.enter_context(tile.TileContext(nc))
        consts = ctx.enter_context(tc.tile_pool(name="consts", bufs=1))
        xpool = ctx.enter_context(tc.tile_pool(name="xc", bufs=2))
        opool = ctx.enter_context(tc.tile_pool(name="ostage", bufs=2))
        spool = ctx.enter_context(tc.tile_pool(name="state", bufs=4))
        upool = ctx.enter_context(tc.tile_pool(name="u", bufs=3))
        gpool = ctx.enter_context(tc.tile_pool(name="gates", bufs=3))
        psum = ctx.enter_context(tc.tile_pool(name="psum", bufs=2, space="PSUM"))

        wblob_sb = consts.tile([DD, 9 * DD], BF, tag="wblob")
        nc.sync.dma_start(out=wblob_sb, in_=wblob[:, :])
        bblob_sb = consts.tile([1, 4 * DD], BF, tag="bblob")
        tb_sb = consts.tile([1, T, DD], BF, tag="tb")
        TBG = T // 16  # tb DMA granule: chunk 0 upfront, rest streamed

        def tb_dma(g):
            nc.sync.dma_start(
                out=tb_sb[:, g * TBG : (g + 1) * TBG, :],
                in_=tb[:, g * TBG : (g + 1) * TBG, :],
            )

        wx_sb = wblob_sb[:, 0 * DD : 1 * DD]
        ws_sb = wblob_sb[:, 1 * DD : 2 * DD]
        wiz_sb = wblob_sb[:, 2 * DD : 3 * DD]
        whz_sb = wblob_sb[:, 3 * DD : 4 * DD]
        wir_sb = wblob_sb[:, 4 * DD : 5 * DD]
        whr_sb = wblob_sb[:, 5 * DD : 6 * DD]
        win_sb = wblob_sb[:, 6 * DD : 7 * DD]
        whn_sb = wblob_sb[:, 7 * DD : 8 * DD]
        ident_sb = wblob_sb[:, 8 * DD : 9 * DD]
        bz_sb = bblob_sb[0:1, 0 * DD : 1 * DD]
        br_sb = bblob_sb[0:1, 1 * DD : 2 * DD]
        bin_sb = bblob_sb[0:1, 2 * DD : 3 * DD]
        bhn_sb = bblob_sb[0:1, 3 * DD : 4 * DD]
        ones_sb = consts.tile([1, HALF], BF)
        nc.vector.memset(ones_sb, 1.0)

        # --- pipeline state per chain ---
        R = []
        for p in range(NPB):
            st = spool.tile([DD, PF[p]], BF, tag=f"state{p}")
            nc.sync.dma_start(out=st, in_=s0[:, POFF[p] : POFF[p + 1]])
            R.append({"s": st})
        nc.sync.dma_start(out=bblob_sb, in_=bblob[:, :])
        tb_dma(0)

        chunks = {}  # chunk idx -> xc tile
        ostages = {}  # chunk idx -> ostage tile

        def get_chunk(c):
            if c not in chunks:
                t0 = c * CH
                xc = xpool.tile([DD, CH, HALF], BF, tag="xc", name="xc")
                nc.sync.dma_start(
                    out=xc, in_=xP[t0 : t0 + CH, :, :].rearrange("c p r -> p c r")
                )
                chunks[c] = xc
            return chunks[c]

        def get_ostage(c):
            if c not in ostages:
                ostages[c] = opool.tile([DD, CH, HALF], BF, tag="ostage", name="ostage")
            return ostages[c]

        def regions(bank, F):
            return (bank[:, 0:F], bank[:, F : 3 * F], bank[:, F : 2 * F],
                    bank[:, 2 * F : 3 * F], bank[:, 3 * F : 4 * F],
                    bank[:, 4 * F : 5 * F])

        def prehoist(p, t):
            """Fresh PSUM bank for (p, t): tb broadcast (start=True, zeroes
            the bank) + Wx@x.  Emitted one phase before A(t)."""
            F = PF[p]
            cs = slice(POFF[p], POFF[p + 1])
            bank = psum.tile([DD, 512], F32, tag=f"bank{p}", name=f"bank{p}")
            pm = bank[:, 0:F]
            nc.tensor.matmul(
                pm, tb_sb[:, t, :], ones_sb[:, 0:F], start=True, stop=False
            )
            xc = get_chunk(t // CH)
            nc.tensor.matmul(pm, wx_sb, xc[:, t % CH, cs], start=False, stop=False)
            R[p]["bank"] = bank

        def phaseA(p, t):
            F = PF[p]
            if p == 0 and t % TBG == TBG // 2 and t + TBG < T:
                tb_dma(t // TBG + 1)
            s_b = R[p]["s"]
            R[p]["bank_cur"] = R[p]["bank"]
            bank = R[p]["bank_cur"]
            pm, zr, zz, rr, pw, hh = regions(bank, F)
            if t == 0:
                nc.tensor.matmul(pm, ws_sb, s_b, start=False, stop=False)
            nc.tensor.matmul(zz, bz_sb, ones_sb[:, 0:F], start=False, stop=False)
            nc.tensor.matmul(rr, br_sb, ones_sb[:, 0:F], start=False, stop=False)
            nc.tensor.matmul(zz, whz_sb, s_b, start=False, stop=False)
            nc.tensor.matmul(rr, whr_sb, s_b, start=False, stop=False)
            nc.tensor.matmul(hh, bhn_sb, ones_sb[:, 0:F], start=False, stop=False)
            nc.tensor.matmul(hh, whn_sb, s_b, start=False, stop=False)
            nc.tensor.matmul(pw, bin_sb, ones_sb[:, 0:F], start=False, stop=False)
            u = upool.tile([DD, F], BF, tag=f"u{p}", name=f"u{p}")
            nc.scalar.activation(u, pm, AF.Gelu)
            R[p]["u"] = u

        def phaseB(p, t):
            F = PF[p]
            s_b = R[p]["s"]
            bank = R[p]["bank_cur"]
            pm, zr, zz, rr, pw, hh = regions(bank, F)
            u = R[p]["u"]
            nc.tensor.matmul(zz, wiz_sb, u, start=False, stop=False)
            nc.tensor.matmul(rr, wir_sb, u, start=False, stop=False)
            nc.tensor.matmul(pw, win_sb, u, start=False, stop=False)
            hzr = gpool.tile([DD, 2 * F], BF, tag=f"hzr{p}", name=f"hzr{p}")
            nc.scalar.activation(hzr, zr, AF.Tanh)
            hz = hzr[:, 0:F]
            hr = hzr[:, F : 2 * F]
            # q = (hr + 1) * hh  (DVE; GPSIMD cannot access PSUM)
            q = gpool.tile([DD, F], BF, tag=f"q{p}", name=f"q{p}")
            nc.vector.scalar_tensor_tensor(q, hr, 1.0, hh, OP.add, OP.mult)
            nc.tensor.matmul(pw, ident_sb, q, start=False, stop=True)
            # blend coefficients: Q = 1-z, Q' = z;  w1 = z*s
            Q = gpool.tile([DD, F], BF, tag=f"Q{p}", name=f"Qc{p}")
            nc.vector.tensor_scalar(Q, hz, 0.5, 0.5, OP.mult, OP.add)
            Qp = gpool.tile([DD, F], BF, tag=f"Qp{p}", name=f"Qp{p}")
            nc.vector.tensor_scalar(Qp, hz, -0.5, 0.5, OP.mult, OP.add)
            w1 = gpool.tile([DD, F], BF, tag=f"w1{p}", name=f"w1{p}")
            nc.vector.tensor_mul(w1, Qp, s_b)
            R[p]["Q"] = Q
            R[p]["w1"] = w1
            # prehoist step t+1's x-side matmuls, then Ws@w1 (Ws@s' is
            # split linearly: Ws@s' = Ws@v + Ws@w1, so the gelu for t+1
            # never waits on s' itself)
            if t + 1 < T:
                prehoist(p, t + 1)
                nbank = R[p]["bank"]
                nc.tensor.matmul(
                    nbank[:, 0:F], ws_sb, w1, start=False, stop=False
                )

        def phaseC(p, t):
            F = PF[p]
            bank = R[p]["bank_cur"]
            pw = bank[:, 3 * F : 4 * F]
            n = gpool.tile([DD, F], BF, tag=f"n{p}", name=f"n{p}")
            nc.scalar.activation(n, pw, AF.Tanh)
            R[p]["n"] = n

        def phaseD(p, t):
            F = PF[p]
            cs = slice(POFF[p], POFF[p + 1])
            n = R[p]["n"]
            v = gpool.tile([DD, F], BF, tag=f"v{p}", name=f"v{p}")
            nc.vector.tensor_mul(v, R[p]["Q"], n)
            if t + 1 < T:
                nbank = R[p]["bank"]
                nc.tensor.matmul(nbank[:, 0:F], ws_sb, v, start=False, stop=False)
            s_nxt = spool.tile([DD, F], BF, tag=f"state{p}", name=f"state{p}")
            nc.vector.tensor_add(s_nxt, v, R[p]["w1"])
            ost = get_ostage(t // CH)
            nc.gpsimd.tensor_copy(out=ost[:, t % CH, cs], in_=s_nxt)
            R[p]["s"] = s_nxt
            if p == NPB - 1 and t % CH == CH - 1:
                c = t // CH
                nc.sync.dma_start(
                    out=outP[t - CH + 1 : t + 1, :, :].rearrange("c p r -> p c r"),
                    in_=ostages.pop(c),
                )
                chunks.pop(c, None)

        # --- bootstrap: bank(0) + x-mms for every chain ---
        for p in range(NPB):
            prehoist(p, 0)

        # --- pipelined slot loop: chain p does phase (i - p) % 4 ---
        PHASES = [phaseA, phaseB, phaseC, phaseD]
        for i in range(4 * T + 3):
            for p in range(NPB):
                ph = (i - p) % 4
                t = (i - p) // 4
                if 0 <= t < T:
                    PHASES[ph](p, t)

    nc.compile()
    return nc


def _blkdiag(a):
    """[64, 64] -> [128, 128] block-diagonal double."""
    out = np.zeros((2 * D, 2 * D), np.float32)
    out[:D, :D] = a
    out[D:, D:] = a
    return out


def _prep_host(x, mask, msg_W, msg_b, W_ih, W_hh, b_ih, b_hh, basis_freq, phase):
    """Host-side prep: partition-packing, sharding, weight doubling."""
    x = np.asarray(x, dtype=np.float32)
    mask = np.asarray(mask)
    msg_W = np.asarray(msg_W, np.float32)
    msg_b = np.asarray(msg_b, np.float32)
    W_ih = np.asarray(W_ih, np.float32)
    W_hh = np.asarray(W_hh, np.float32)
    b_ih = np.asarray(b_ih, np.float32)
    b_hh = np.asarray(b_hh, np.float32)
    basis_freq = np.asarray(basis_freq, np.float32)
    phase = np.asarray(phase, np.float32)

    tr = np.arange(T, dtype=np.int64) * mask.astype(np.int64)
    identity_gather = bool(np.array_equal(tr, np.arange(T)))

    xf = x.reshape(T, B * C, D)
    s0_rows = xf.mean(axis=0)  # [B*C, D] f32 (from ungathered x)
    if not identity_gather:
        xf = xf[tr]

    xP8, s08 = [], []
    for c in range(NCORES):
        blk = xf[:, c * ROWS : (c + 1) * ROWS, :]  # [T, 512, 64]
        lo = blk[:, 0:HALF].transpose(0, 2, 1)  # [T, 64, 256]
        hi = blk[:, HALF:ROWS].transpose(0, 2, 1)
        xP8.append(np.ascontiguousarray(
            np.concatenate([lo, hi], axis=1)).astype(BF16))
        sblk = s0_rows[c * ROWS : (c + 1) * ROWS]  # [512, 64]
        s08.append(np.ascontiguousarray(np.concatenate(
            [sblk[0:HALF].T, sblk[HALF:ROWS].T], axis=0)).astype(BF16))

    ts_ = np.arange(T, dtype=np.float32)[tr]
    te = np.cos(ts_[:, None] * basis_freq[None, :] + phase[None, :])  # [T, D]
    Wt = msg_W[:, 2 * D : 3 * D]
    tb1 = te @ Wt.T + msg_b[None, :]  # [T, 64]
    tb_host = np.tile(tb1, (1, 2)).astype(BF16).reshape(1, T, 2 * D)

    Wx = msg_W[:, 0:D].T
    Ws = msg_W[:, D : 2 * D].T
    # torch gate order in W_ih/W_hh: rows [r, z, n]
    Wir, Wiz, Win = W_ih[0:D], W_ih[D : 2 * D], W_ih[2 * D : 3 * D]
    Whr, Whz, Whn = W_hh[0:D], W_hh[D : 2 * D], W_hh[2 * D : 3 * D]

    mats = [
        Wx, Ws,
        -0.5 * Wiz.T, -0.5 * Whz.T,
        0.5 * Wir.T, 0.5 * Whr.T,
        Win.T, 0.5 * Whn.T,
        np.eye(D, dtype=np.float32),
    ]
    wblob = np.concatenate([_blkdiag(m) for m in mats], axis=1)

    bblob = np.concatenate([
        np.tile(-0.5 * (b_ih[D : 2 * D] + b_hh[D : 2 * D]), 2),
        np.tile(0.5 * (b_ih[0:D] + b_hh[0:D]), 2),
        np.tile(b_ih[2 * D : 3 * D], 2),
        np.tile(0.5 * b_hh[2 * D : 3 * D], 2),
    ]).reshape(1, 4 * 2 * D)

    shared = {
        "tb": tb_host,
        "wblob": wblob.astype(BF16),
        "bblob": bblob.astype(BF16),
    }
    in_maps = []
    for c in range(NCORES):
        m = dict(shared)
        m["xP"] = xP8[c]
        m["s0"] = s08[c]
        in_maps.append(m)
    return in_maps


def kernel(**inputs):
    from concourse.bass_utils import run_bass_kernel_spmd

    in_maps = _prep_host(**inputs)

    if "prog" not in _PROGRAM_CACHE:
        _PROGRAM_CACHE["prog"] = _build_program()
    nc = _PROGRAM_CACHE["prog"]

    res = run_bass_kernel_spmd(nc, in_maps, core_ids=list(range(NCORES)))
    _PROGRAM_CACHE["last_results"] = res

    out = np.empty((T, B * C, D), dtype=np.float32)
    for c in range(NCORES):
        outP_c = np.asarray(res.results[c]["outP"], dtype=np.float32)  # [T,128,256]
        base = c * ROWS
        out[:, base : base + HALF, :] = outP_c[:, 0:D, :].transpose(0, 2, 1)
        out[:, base + HALF : base + ROWS, :] = outP_c[:, D:, :].transpose(0, 2, 1)
    return out.reshape(T, B, C, D)


# revision 16
# speedup vs baseline: 1.0102x; 1.0102x over previous
"""Trainium2 Bass kernel for nn_MemoryNetwork (GRU-style memory network scan).

Model (per reference):
  t_enc = cos(arange(T) * freq + phase)                    [T, D]
  s0 = mean_t(x)                                           [B*C, D]
  tr = arange(T) * mask; x_seq = x[tr]; te_seq = t_enc[tr]
  per step t:
    msg = gelu([x_t, s, te_t] @ msg_W.T + msg_b)
    gi = msg @ W_ih.T + b_ih ; gh = s @ W_hh.T + b_hh
    r = sigmoid(i_r + h_r); z = sigmoid(i_z + h_z)
    n = tanh(i_n + r * h_n)
    s' = (1 - z) * n + z * s
  output: states [T, B, C, D]

Strategy: data-parallel over B*C = 4096 rows -> 8 cores x 512 rows.
On-device layout is feature-major and PARTITION-PACKED: two rows share a
packed column (row c on partitions 0:64, row 256+c on partitions 64:128),
with block-diagonal doubled weights [128, 128].  Matmul cost depends only
on the free dim, so packing halves the activation-engine work (the
bottleneck) for free.  The 256 packed columns split into NPB=3
independent chains (86/85/85).

Engines execute their queues IN PROGRAM ORDER, so the emission order is
SOFTWARE-PIPELINED: chain p runs phase (slot - p) % 3 of its step at each
slot, staggering the three chains across the three ACT ops of a step
(gelu / zr-tanh / n-tanh).  Between two consecutive ACT ops of one chain
the ACT engine executes the other two chains' (independent) ops, hiding
each chain's PE/DVE dependency latency.  Phases:

  A(t): s-dependent matmuls (Ws@s into pm; Whz/Whr@s + z/r bias rows into
        zr; Whn@s + bias into hh; b_in into pw), then u = gelu(pm).
  B(t): u-side matmuls (Wiz/Wir@u, Win@u), hzr = tanh([-a_z/2 | +a_r/2]),
        q = (hr+1)*hh on DVE, I@q accumulated into pw (PE),
        Q = 0.5*hz+0.5, Q' = -0.5*hz+0.5 (= 1-z, z), w1 = Q'*s on DVE.
  C(t): n = tanh(pw), v = Q*n, s' = v + w1 (= (1-z)*n + z*s), Pool-copy
        s' to the output stage, and prehoist step t+1's x-dependent
        matmuls (tb broadcast row with start=True, Wx@x) into a fresh
        PSUM bank.

All gate scales/biases are folded into the doubled weights or added in
PSUM via broadcast-row matmuls, so every ACT op is plain func(x).  Each
chain-step owns one PSUM bank [128, 512] f32 (one start=True / one
stop=True per bank: start zeroes the whole 2KB zero-region).  Outputs
stream out as bf16 every CH steps; the host unpacks partitions to rows.
"""

import sys

import numpy as np

sys.path.insert(0, "/opt/trn_rl_repo")

import ml_dtypes  # noqa: E402

BF16 = ml_dtypes.bfloat16

T, B, C, D = 256, 64, 64, 64
NCORES = 8
ROWS = (B * C) // NCORES  # 512 rows per core
HALF = ROWS // 2  # 256 packed columns (2 rows per column)
CH = 8  # timesteps per DMA chunk
NPB = 3  # packed chains per core
PF = [86, 85, 85]  # packed columns per chain
POFF = [0, 86, 171, 256]

_PROGRAM_CACHE = {}


def _build_program():
    import concourse.bacc as bacc
    import concourse.tile as tile
    from concourse import mybir
    from contextlib import ExitStack

    BF = mybir.dt.bfloat16
    F32 = mybir.dt.float32
    AF = mybir.ActivationFunctionType
    OP = mybir.AluOpType

    nc = bacc.Bacc(None, target_bir_lowering=False, debug=False)

    xP = nc.dram_tensor("xP", [T, 2 * D, HALF], BF, kind="ExternalInput")
    s0 = nc.dram_tensor("s0", [2 * D, HALF], BF, kind="ExternalInput")
    tb = nc.dram_tensor("tb", [1, T, 2 * D], BF, kind="ExternalInput")
    # 8 block-diagonal doubled weight mats + identity, packed [128, 9*128]:
    #   wx, ws, wiz(-.5), whz(-.5), wir(+.5), whr(+.5), win, whn(.5), I
    wblob = nc.dram_tensor("wblob", [2 * D, 9 * 2 * D], BF, kind="ExternalInput")
    # bias rows on partition 0, side by side [1, 4*128]:
    # -0.5(b_iz+b_hz) | +0.5(b_ir+b_hr) | b_in | 0.5*b_hn  (each tiled 2x)
    bblob = nc.dram_tensor("bblob", [1, 4 * 2 * D], BF, kind="ExternalInput")
    outP = nc.dram_tensor("outP", [T, 2 * D, HALF], BF, kind="ExternalOutput")

    DD = 2 * D  # 128

    with ExitStack() as ctx:
        tc = ctx.enter_context(tile.TileContext(nc))
        consts = ctx.enter_context(tc.tile_pool(name="consts", bufs=1))
        xpool = ctx.enter_context(tc.tile_pool(name="xc", bufs=2))
        opool = ctx.enter_context(tc.tile_pool(name="ostage", bufs=2))
        spool = ctx.enter_context(tc.tile_pool(name="state", bufs=4))
        upool = ctx.enter_context(tc.tile_pool(name="u", bufs=3))
        gpool = ctx.enter_context(tc.tile_pool(name="gates", bufs=3))
        psum = ctx.enter_context(tc.tile_pool(name="psum", bufs=2, space="PSUM"))

        wblob_sb = consts.tile([DD, 9 * DD], BF, tag="wblob")
        nc.sync.dma_start(out=wblob_sb, in_=wblob[:, :])
        bblob_sb = consts.tile([1, 4 * DD], BF, tag="bblob")
        tb_sb = consts.tile([1, T, DD], BF, tag="tb")
        TBG = T // 8  # tb DMA granule: chunk 0 upfront, rest streamed

        def tb_dma(g):
            nc.sync.dma_start(
                out=tb_sb[:, g * TBG : (g + 1) * TBG, :],
                in_=tb[:, g * TBG : (g + 1) * TBG, :],
            )

        wx_sb = wblob_sb[:, 0 * DD : 1 * DD]
        ws_sb = wblob_sb[:, 1 * DD : 2 * DD]
        wiz_sb = wblob_sb[:, 2 * DD : 3 * DD]
        whz_sb = wblob_sb[:, 3 * DD : 4 * DD]
        wir_sb = wblob_sb[:, 4 * DD : 5 * DD]
        whr_sb = wblob_sb[:, 5 * DD : 6 * DD]
        win_sb = wblob_sb[:, 6 * DD : 7 * DD]
        whn_sb = wblob_sb[:, 7 * DD : 8 * DD]
        ident_sb = wblob_sb[:, 8 * DD : 9 * DD]
        bz_sb = bblob_sb[0:1, 0 * DD : 1 * DD]
        br_sb = bblob_sb[0:1, 1 * DD : 2 * DD]
        bin_sb = bblob_sb[0:1, 2 * DD : 3 * DD]
        bhn_sb = bblob_sb[0:1, 3 * DD : 4 * DD]
        ones_sb = consts.tile([1, HALF], BF)
        nc.vector.memset(ones_sb, 1.0)

        # --- pipeline state per chain ---
        R = []
        for p in range(NPB):
            st = spool.tile([DD, PF[p]], BF, tag=f"state{p}")
            nc.sync.dma_start(out=st, in_=s0[:, POFF[p] : POFF[p + 1]])
            R.append({"s": st})
        nc.sync.dma_start(out=bblob_sb, in_=bblob[:, :])
        tb_dma(0)

        chunks = {}  # chunk idx -> xc tile
        ostages = {}  # chunk idx -> ostage tile

        def get_chunk(c):
            if c not in chunks:
                t0 = c * CH
                xc = xpool.tile([DD, CH, HALF], BF, tag="xc", name="xc")
                nc.sync.dma_start(
                    out=xc, in_=xP[t0 : t0 + CH, :, :].rearrange("c p r -> p c r")
                )
                chunks[c] = xc
            return chunks[c]

        def get_ostage(c):
            if c not in ostages:
                ostages[c] = opool.tile([DD, CH, HALF], BF, tag="ostage", name="ostage")
            return ostages[c]

        def regions(bank, F):
            return (bank[:, 0:F], bank[:, F : 3 * F], bank[:, F : 2 * F],
                    bank[:, 2 * F : 3 * F], bank[:, 3 * F : 4 * F],
                    bank[:, 4 * F : 5 * F])

        def prehoist(p, t):
            """Fresh PSUM bank for (p, t): tb broadcast (start=True, zeroes
            the bank) + Wx@x.  Emitted one phase before A(t)."""
            F = PF[p]
            cs = slice(POFF[p], POFF[p + 1])
            bank = psum.tile([DD, 512], F32, tag=f"bank{p}", name=f"bank{p}")
            pm = bank[:, 0:F]
            nc.tensor.matmul(
                pm, tb_sb[:, t, :], ones_sb[:, 0:F], start=True, stop=False
            )
            xc = get_chunk(t // CH)
            nc.tensor.matmul(pm, wx_sb, xc[:, t % CH, cs], start=False, stop=False)
            R[p]["bank"] = bank

        def phaseA(p, t):
            F = PF[p]
            if p == 0 and t % TBG == TBG // 2 and t + TBG < T:
                tb_dma(t // TBG + 1)
            s_b = R[p]["s"]
            R[p]["bank_cur"] = R[p]["bank"]
            bank = R[p]["bank_cur"]
            pm, zr, zz, rr, pw, hh = regions(bank, F)
            if t == 0:
                nc.tensor.matmul(pm, ws_sb, s_b, start=False, stop=False)
            nc.tensor.matmul(zz, bz_sb, ones_sb[:, 0:F], start=False, stop=False)
            nc.tensor.matmul(rr, br_sb, ones_sb[:, 0:F], start=False, stop=False)
            nc.tensor.matmul(zz, whz_sb, s_b, start=False, stop=False)
            nc.tensor.matmul(rr, whr_sb, s_b, start=False, stop=False)
            nc.tensor.matmul(hh, bhn_sb, ones_sb[:, 0:F], start=False, stop=False)
            nc.tensor.matmul(hh, whn_sb, s_b, start=False, stop=False)
            nc.tensor.matmul(pw, bin_sb, ones_sb[:, 0:F], start=False, stop=False)
            u = upool.tile([DD, F], BF, tag=f"u{p}", name=f"u{p}")
            nc.scalar.activation(u, pm, AF.Gelu)
            R[p]["u"] = u

        def phaseB(p, t):
            F = PF[p]
            s_b = R[p]["s"]
            bank = R[p]["bank_cur"]
            pm, zr, zz, rr, pw, hh = regions(bank, F)
            u = R[p]["u"]
            nc.tensor.matmul(zz, wiz_sb, u, start=False, stop=False)
            nc.tensor.matmul(rr, wir_sb, u, start=False, stop=False)
            nc.tensor.matmul(pw, win_sb, u, start=False, stop=False)
            hzr = gpool.tile([DD, 2 * F], BF, tag=f"hzr{p}", name=f"hzr{p}")
            nc.scalar.activation(hzr, zr, AF.Tanh)
            hz = hzr[:, 0:F]
            hr = hzr[:, F : 2 * F]
            # q = (hr + 1) * hh  (DVE; GPSIMD cannot access PSUM)
            q = gpool.tile([DD, F], BF, tag=f"q{p}", name=f"q{p}")
            nc.vector.scalar_tensor_tensor(q, hr, 1.0, hh, OP.add, OP.mult)
            nc.tensor.matmul(pw, ident_sb, q, start=False, stop=True)
            # blend coefficients: Q = 1-z, Q' = z;  w1 = z*s
            Q = gpool.tile([DD, F], BF, tag=f"Q{p}", name=f"Qc{p}")
            nc.vector.tensor_scalar(Q, hz, 0.5, 0.5, OP.mult, OP.add)
            Qp = gpool.tile([DD, F], BF, tag=f"Qp{p}", name=f"Qp{p}")
            nc.vector.tensor_scalar(Qp, hz, -0.5, 0.5, OP.mult, OP.add)
            w1 = gpool.tile([DD, F], BF, tag=f"w1{p}", name=f"w1{p}")
            nc.vector.tensor_mul(w1, Qp, s_b)
            R[p]["Q"] = Q
            R[p]["w1"] = w1
            # prehoist step t+1's x-side matmuls, then Ws@w1 (Ws@s' is
            # split linearly: Ws@s' = Ws@v + Ws@w1, so the gelu for t+1
            # never waits on s' itself)
            if t + 1 < T:
                prehoist(p, t + 1)
                nbank = R[p]["bank"]
                nc.tensor.matmul(
                    nbank[:, 0:F], ws_sb, w1, start=False, stop=False
                )

        def phaseC(p, t):
            F = PF[p]
            bank = R[p]["bank_cur"]
            pw = bank[:, 3 * F : 4 * F]
            n = gpool.tile([DD, F], BF, tag=f"n{p}", name=f"n{p}")
            nc.scalar.activation(n, pw, AF.Tanh)
            R[p]["n"] = n

        def phaseD(p, t):
            F = PF[p]
            cs = slice(POFF[p], POFF[p + 1])
            n = R[p]["n"]
            v = gpool.tile([DD, F], BF, tag=f"v{p}", name=f"v{p}")
            nc.vector.tensor_mul(v, R[p]["Q"], n)
            if t + 1 < T:
                nbank = R[p]["bank"]
                nc.tensor.matmul(nbank[:, 0:F], ws_sb, v, start=False, stop=False)
            s_nxt = spool.tile([DD, F], BF, tag=f"state{p}", name=f"state{p}")
            nc.vector.tensor_add(s_nxt, v, R[p]["w1"])
            ost = get_ostage(t // CH)
            nc.gpsimd.tensor_copy(out=ost[:, t % CH, cs], in_=s_nxt)
            R[p]["s"] = s_nxt
            if p == NPB - 1 and t % CH == CH - 1:
                c = t // CH
                nc.sync.dma_start(
                    out=outP[t - CH + 1 : t + 1, :, :].rearrange("c p r -> p c r"),
                    in_=ostages.pop(c),
                )
                chunks.pop(c, None)

        # --- bootstrap: bank(0) + x-mms for every chain ---
        for p in range(NPB):
            prehoist(p, 0)

        # --- pipelined slot loop: chain p does phase (i - p) % 4 ---
        PHASES = [phaseA, phaseB, phaseC, phaseD]
        for i in range(4 * T + 3):
            for p in range(NPB):
                ph = (i - p) % 4
                t = (i - p) // 4
                if 0 <= t < T:
                    PHASES[ph](p, t)

    nc.compile()
    return nc


def _blkdiag(a):
    """[64, 64] -> [128, 128] block-diagonal double."""
    out = np.zeros((2 * D, 2 * D), np.float32)
    out[:D, :D] = a
    out[D:, D:] = a
    return out


def _prep_host(x, mask, msg_W, msg_b, W_ih, W_hh, b_ih, b_hh, basis_freq, phase):
    """Host-side prep: partition-packing, sharding, weight doubling."""
    x = np.asarray(x, dtype=np.float32)
    mask = np.asarray(mask)
    msg_W = np.asarray(msg_W, np.float32)
    msg_b = np.asarray(msg_b, np.float32)
    W_ih = np.asarray(W_ih, np.float32)
    W_hh = np.asarray(W_hh, np.float32)
    b_ih = np.asarray(b_ih, np.float32)
    b_hh = np.asarray(b_hh, np.float32)
    basis_freq = np.asarray(basis_freq, np.float32)
    phase = np.asarray(phase, np.float32)

    tr = np.arange(T, dtype=np.int64) * mask.astype(np.int64)
    identity_gather = bool(np.array_equal(tr, np.arange(T)))

    xf = x.reshape(T, B * C, D)
    s0_rows = xf.mean(axis=0)  # [B*C, D] f32 (from ungathered x)
    if not identity_gather:
        xf = xf[tr]

    xP8, s08 = [], []
    for c in range(NCORES):
        blk = xf[:, c * ROWS : (c + 1) * ROWS, :]  # [T, 512, 64]
        lo = blk[:, 0:HALF].transpose(0, 2, 1)  # [T, 64, 256]
        hi = blk[:, HALF:ROWS].transpose(0, 2, 1)
        xP8.append(np.ascontiguousarray(
            np.concatenate([lo, hi], axis=1)).astype(BF16))
        sblk = s0_rows[c * ROWS : (c + 1) * ROWS]  # [512, 64]
        s08.append(np.ascontiguousarray(np.concatenate(
            [sblk[0:HALF].T, sblk[HALF:ROWS].T], axis=0)).astype(BF16))

    ts_ = np.arange(T, dtype=np.float32)[tr]
    te = np.cos(ts_[:, None] * basis_freq[None, :] + phase[None, :])  # [T, D]
    Wt = msg_W[:, 2 * D : 3 * D]
    tb1 = te @ Wt.T + msg_b[None, :]  # [T, 64]
    tb_host = np.tile(tb1, (1, 2)).astype(BF16).reshape(1, T, 2 * D)

    Wx = msg_W[:, 0:D].T
    Ws = msg_W[:, D : 2 * D].T
    # torch gate order in W_ih/W_hh: rows [r, z, n]
    Wir, Wiz, Win = W_ih[0:D], W_ih[D : 2 * D], W_ih[2 * D : 3 * D]
    Whr, Whz, Whn = W_hh[0:D], W_hh[D : 2 * D], W_hh[2 * D : 3 * D]

    mats = [
        Wx, Ws,
        -0.5 * Wiz.T, -0.5 * Whz.T,
        0.5 * Wir.T, 0.5 * Whr.T,
        Win.T, 0.5 * Whn.T,
        np.eye(D, dtype=np.float32),
    ]
    wblob = np.concatenate([_blkdiag(m) for m in mats], axis=1)

    bblob = np.concatenate([
        np.tile(-0.5 * (b_ih[D : 2 * D] + b_hh[D : 2 * D]), 2),
        np.tile(0.5 * (b_ih[0:D] + b_hh[0:D]), 2),
        np.tile(b_ih[2 * D : 3 * D], 2),
        np.tile(0.5 * b_hh[2 * D : 3 * D], 2),
    ]).reshape(1, 4 * 2 * D)

    shared = {
        "tb": tb_host,
        "wblob": wblob.astype(BF16),
        "bblob": bblob.astype(BF16),
    }
    in_maps = []
    for c in range(NCORES):
        m = dict(shared)
        m["xP"] = xP8[c]
        m["s0"] = s08[c]
        in_maps.append(m)
    return in_maps


def kernel(**inputs):
    from concourse.bass_utils import run_bass_kernel_spmd

    in_maps = _prep_host(**inputs)

    if "prog" not in _PROGRAM_CACHE:
        _PROGRAM_CACHE["prog"] = _build_program()
    nc = _PROGRAM_CACHE["prog"]

    res = run_bass_kernel_spmd(nc, in_maps, core_ids=list(range(NCORES)))
    _PROGRAM_CACHE["last_results"] = res

    out = np.empty((T, B * C, D), dtype=np.float32)
    for c in range(NCORES):
        outP_c = np.asarray(res.results[c]["outP"], dtype=np.float32)  # [T,128,256]
        base = c * ROWS
        out[:, base : base + HALF, :] = outP_c[:, 0:D, :].transpose(0, 2, 1)
        out[:, base + HALF : base + ROWS, :] = outP_c[:, D:, :].transpose(0, 2, 1)
    return out.reshape(T, B, C, D)
